# revision 12
# baseline (speedup 1.0000x reference)
"""CaNet (moe_routing GNN) forward on 8 Trainium2 NeuronCores.

Sharding: nodes range-partitioned across 8 cores (6250 each, padded to
6272 = 49*128). Each core owns edges whose destination lands in its range.
GCN aggregation out[col] += val * h[row] runs as one-hot matmuls:

  - edges sorted by destination tile (128 dest nodes), padded to 128-chunks;
  - source features fetched from a replicated node-major bf16 table in HBM
    via GpSimd dma_gather; the table splits into "lo" (slab rows [0,3968),
    8*3968=31744 rows) and "hi" (rows [3968,6272), 8*2304=18432 rows) halves
    so every row index fits int16;
  - gather calls round-robin all 4 SWDGE queues (each queue runs on its own
    Q7 core pair, so 4 desc-gens proceed concurrently); G-tile pool depth 5
    per stream keeps the queues fed ahead of consumption;
  - prebuilt selection matrices S ([128e x 128d] bf16, S[e,d]=(d==dest)*val)
    stream from HBM in 32-chunk windows on the HWDGE sync queue;
  - psum_gcnT[f,d] += G_chunk.T @ S accumulates per dest tile.

Dense work per 128-node tile is restructured to keep the Vector engine off
the critical path (the gather desc-gen on GpSimd is the pacing resource):
  - residual is folded into the expert weights (wbot' = wbot + I), so
    relu(mix + h) becomes relu(sum_k e_k * po'_k / esum);
  - the expert-gate softmax is computed per layer in bulk: one psum holds
    all 49 tiles' logits, one Vector add applies env_b, one negated
    reduce_max + per-tile scalar-engine Exp with accum_out (row sums) and
    one Vector reciprocal produce unnormalized gates + 1/sum scales;
  - mixing uses 2 scalar-engine + 2 vector scaled copies and 2 vector adds;
    the 1/sum lands as the scale of the final scalar-engine Relu.

Between layers h is AllGather'd in two pieces (lo as soon as tile 30 is
done, hi after tile 48); collectives are emitted so they never head-of-line
block pending gather calls in the in-order GpSimd queue, and bulk DMAs
(agin/out) ride the scalar-engine HWDGE queue instead of sync so S-window
backpressure cannot delay them.
"""

import sys

sys.path.insert(0, "/opt/trn_rl_repo")

import numpy as np
import ml_dtypes

import concourse.bacc as bacc
import concourse.tile as tile
import concourse.mybir as mybir
import concourse.bass as bass
from concourse import bass_utils
from concourse.masks import make_identity

# Problem constants (hardcoded per contract).
N = 50000
E = 800000
D = 128  # input dim
H = 128  # hidden dim
C = 47  # classes
K = 4  # experts
L = 2  # conv layers
M = 8  # cores

NPC = N // M  # 6250 nodes per core
T = (NPC + 127) // 128  # 49 tiles per core
NPAD = T * 128  # 6272
TSPL = 31  # tiles in the "lo" slab half (max with 8*SPL <= int16 range)
SPL = TSPL * 128  # 3968
SPH = NPAD - SPL  # 2304
LO = M * SPL  # 31744 lo-table rows (int16-safe)
HI = M * SPH  # 18432 hi-table rows
CALL = 32  # chunks per full dma_gather call (4096 indices)
SWIN = 32  # chunks per streamed S-matrix window

F32 = mybir.dt.float32
BF16 = mybir.dt.bfloat16
I16 = mybir.dt.int16
BF = ml_dtypes.bfloat16


def _preprocess(x, edge_index, fc0_w, fc0_b, fc1_w, fc1_b, env_w, env_b, conv_w):
    """Host-side: degree/value computation, edge sort, static chunk schedule,
    per-core gather/selection arrays, weight packing."""
    row = np.asarray(edge_index[0], np.int64)
    col = np.asarray(edge_index[1], np.int64)

    deg = np.bincount(col, minlength=N).astype(np.float32)
    dinv = np.where(deg > 0, 1.0 / np.sqrt(deg), 0.0).astype(np.float32)
    val = (dinv[col] * dinv[row]).astype(np.float32)

    core = col // NPC
    dloc = col % NPC
    tl = dloc // 128
    ld = (dloc % 128).astype(np.float32)
    score = row // NPC  # source core
    soff = row % NPC  # offset within source slab (< NPC <= NPAD)
    half = (soff >= SPL).astype(np.int64)
    # agin buffers are shipped partition-major ([128p, tiles, H]), so table
    # row for node offset off = t*128+p is p*TILES + t within the core block.
    tA = soff // 128
    pA = soff % 128
    tB = (soff - SPL) // 128
    idx16 = np.where(
        half == 1,
        score * SPH + pA * (T - TSPL) + tB,
        score * SPL + pA * TSPL + tA,
    )

    # group = (core, half, tile); count chunks per (tile, half) max'd over cores
    cnt = np.bincount((core * T + tl) * 2 + half, minlength=M * T * 2).reshape(
        M, T, 2
    )
    nch = -(-cnt // 128)  # ceil div, [M, T, 2]
    NCH = nch.max(axis=0)  # [T, 2] static schedule
    assert NCH.sum() > 0
    tot = NCH.sum(axis=0)  # [2] total chunks per stream (no call padding)
    # call sizes: full CALL-chunk calls plus one short final call
    callsz = []
    for s in range(2):
        t_s = int(tot[s])
        cs = [CALL] * (t_s // CALL)
        if t_s % CALL:
            cs.append(t_s % CALL)
        callsz.append(cs)

    # stream-local chunk base per tile (exclusive cumsum), shared across cores
    base = np.zeros((T, 2), np.int64)
    base[1:] = NCH[:-1].cumsum(axis=0)

    # within-group rank for every edge
    gkey = (core * 2 + half) * T + tl
    order = np.argsort(gkey, kind="stable")
    gsort = gkey[order]
    starts = np.searchsorted(gsort, np.arange(M * 2 * T))
    rank = np.arange(E, dtype=np.int64) - starts[gsort]
    # slot within (core, half) stream
    slot = np.empty(E, np.int64)
    slot[order] = base[tl[order], half[order]] * 128 + rank

    idx_arr = np.zeros((M, 2), object)
    v_arr = np.zeros((M, 2), object)
    s_arr = np.zeros((M, 2), object)
    for c in range(M):
        for s in range(2):
            nslot = int(tot[s]) * 128
            ia = np.zeros(nslot, np.int16)
            la = np.zeros(nslot, np.int64)
            va = np.zeros(nslot, np.float32)
            sel = (core == c) & (half == s)
            ia[slot[sel]] = idx16[sel].astype(np.int16)
            la[slot[sel]] = ld[sel].astype(np.int64)
            va[slot[sel]] = val[sel]
            # wrap indices for dma_gather: per call [16, C*8] tiled x8 -> [128, C*8]
            pieces = []
            off = 0
            for cs in callsz[s]:
                ni = cs * 128
                iw = ia[off : off + ni].reshape(ni // 16, 16).T  # [16, cs*8]
                pieces.append(np.tile(iw, (8, 1)))  # [128, cs*8]
                off += ni
            idx_arr[c, s] = np.concatenate(pieces, axis=1)
            v_arr[c, s] = va.reshape(int(tot[s]), 128).T.copy()
            # prebuilt selection matrices: S[e, ch, d] = (d == ld)*val
            sf = np.zeros((nslot, 128), BF)
            sf[np.arange(nslot), la] = va.astype(BF)
            sf[va == 0.0] = 0  # padding slots contribute nothing even if ld=0
            s_arr[c, s] = np.ascontiguousarray(
                sf.reshape(int(tot[s]), 128, 128).transpose(1, 0, 2)
            )

    # x slabs, transposed + padded: [128 d, NPAD n] bf16 per core
    x = np.asarray(x, np.float32)
    xT = np.zeros((M, D, NPAD), BF)
    for c in range(M):
        xT[c, :, :NPC] = x[c * NPC : (c + 1) * NPC].T.astype(BF)

    # packed weights; wbot gets +I so the residual rides the expert matmul
    conv_w = np.asarray(conv_w, np.float32)
    eye = np.eye(H, dtype=np.float32)
    wtop = np.zeros((L, H, K * H), BF)
    wbot = np.zeros((L, H, K * H), BF)
    for l in range(L):
        for k in range(K):
            wtop[l, :, k * H : (k + 1) * H] = conv_w[l, k, :H].astype(BF)
            wbot[l, :, k * H : (k + 1) * H] = (conv_w[l, k, H:] + eye).astype(BF)
    env_w = np.asarray(env_w, np.float32)
    env_b = np.asarray(env_b, np.float32)
    # env bias tiled over [128, T, K] so one vector add applies it per layer
    envb = np.zeros((L, 128, T, K), np.float32)
    for l in range(L):
        envb[l] = np.tile(env_b[l], (128, T, 1))
    prep = dict(
        NCH=NCH,
        base=base,
        callsz=callsz,
        tot=tot,
        idx_arr=idx_arr,
        v_arr=v_arr,
        s_arr=s_arr,
        xT=xT,
        fc0_w=np.asarray(fc0_w, BF),
        b0=np.asarray(fc0_b, np.float32),
        wtop=wtop,
        wbot=wbot,
        env_w_bf=env_w.astype(BF),
        envb=envb,
        fc1_w=np.asarray(fc1_w, BF),
        b1_bcast=np.tile(np.asarray(fc1_b, np.float32), (128, 1)),
    )
    return prep


def _emulate(prep):
    """Numpy mirror of the device program (validates schedule/indexing)."""
    NCH, base, callsz, tot = prep["NCH"], prep["base"], prep["callsz"], prep["tot"]
    h_node = np.zeros((M, NPAD, H), np.float32)
    for c in range(M):
        z = prep["xT"][c].T.astype(np.float32) @ prep["fc0_w"].astype(
            np.float32
        ) + prep["b0"]
        h_node[c] = np.maximum(z, 0.0)

    def tables(hn):
        # p-major within each core: row p*TILES + t = node t*128 + p
        tlo = np.concatenate([
            hn[c, :SPL].reshape(TSPL, 128, H).transpose(1, 0, 2).reshape(SPL, H).astype(BF)
            for c in range(M)
        ])
        thi = np.concatenate([
            hn[c, SPL:].reshape(T - TSPL, 128, H).transpose(1, 0, 2).reshape(SPH, H).astype(BF)
            for c in range(M)
        ])
        return tlo, thi

    tlo, thi = tables(h_node)

    for l in range(L):
        new_h = np.zeros_like(h_node)
        for c in range(M):
            G = [None, None]
            for s in range(2):
                ia = prep["idx_arr"][c, s]
                idxs = []
                off = 0
                for cs in callsz[s]:
                    blkw = ia[:16, off : off + cs * 8]  # [16, cs*8]
                    idxs.append(blkw.T.reshape(-1))
                    off += cs * 8
                idxs = np.concatenate(idxs).astype(np.int64)
                tab = tlo if s == 0 else thi
                G[s] = tab[idxs].astype(np.float32)
            gcnT = np.zeros((T, H, 128), np.float32)
            for t in range(T):
                acc = np.zeros((H, 128), np.float32)
                for s in range(2):
                    for jc in range(NCH[t, s]):
                        ch = base[t, s] + jc
                        g = G[s][ch * 128 : (ch + 1) * 128]
                        S = prep["s_arr"][c, s][:, ch, :].astype(np.float32)
                        acc += g.astype(BF).astype(np.float32).T @ S
                gcnT[t] = acc
            hT_bf = h_node[c].T.astype(BF)
            for t in range(T):
                sl = slice(t * 128, (t + 1) * 128)
                ht = hT_bf[:, sl].astype(np.float32)
                z = ht.T @ prep["env_w_bf"][l].astype(np.float32)
                z = z + prep["envb"][l][0, t][None, :]
                zm = z.max(axis=1, keepdims=True)
                e = np.exp(z - zm)
                recip = 1.0 / e.sum(axis=1, keepdims=True)
                gt = gcnT[t].astype(BF).astype(np.float32)
                O = gt.T @ prep["wtop"][l].astype(np.float32) + ht.T @ prep[
                    "wbot"
                ][l].astype(np.float32)
                O = O.reshape(128, K, H)
                mixed = np.einsum("nk,nkh->nh", e, O)
                new_h[c, sl] = np.maximum(mixed * recip, 0.0)
        h_node = new_h
        tlo, thi = tables(h_node)

    out = np.zeros((N, C), np.float32)
    for c in range(M):
        z = h_node[c] @ prep["fc1_w"].astype(np.float32) + prep["b1_bcast"][0][None, :]
        out[c * NPC : (c + 1) * NPC] = z[:NPC]
    return out


def _build_program(prep):
    NCH, base, callsz, tot = prep["NCH"], prep["base"], prep["callsz"], prep["tot"]
    nc = bacc.Bacc(
        "TRN2",
        target_bir_lowering=False,
        debug=False,
        num_devices=M,
        num_swdge_queues=4,
        dynamic_dma_scratch_size=24576,
    )
    # I/O
    xT = nc.dram_tensor("xT", [D, NPAD], BF16, kind="ExternalInput")
    idx_io = [
        nc.dram_tensor(f"idx{s}", [128, int(tot[s]) * 8], I16, kind="ExternalInput")
        for s in range(2)
    ]
    s_io = [
        nc.dram_tensor(f"smat{s}", [128, int(tot[s]), 128], BF16, kind="ExternalInput")
        for s in range(2)
    ]
    fc0_w = nc.dram_tensor("fc0_w", [D, H], BF16, kind="ExternalInput")
    b0col = nc.dram_tensor("b0col", [H, 1], F32, kind="ExternalInput")
    wtop = nc.dram_tensor("wtop", [L, H, K * H], BF16, kind="ExternalInput")
    wbot = nc.dram_tensor("wbot", [L, H, K * H], BF16, kind="ExternalInput")
    env_w = nc.dram_tensor("env_w", [L, H, K], BF16, kind="ExternalInput")
    envb_io = nc.dram_tensor("envb", [L, 128, T, K], F32, kind="ExternalInput")
    fc1_w = nc.dram_tensor("fc1_w", [H, C], BF16, kind="ExternalInput")
    b1 = nc.dram_tensor("b1", [128, C], F32, kind="ExternalInput")
    out_io = nc.dram_tensor("out", [NPAD, C], F32, kind="ExternalOutput")

    # internal DRAM: split gather tables + per-layer AllGather inputs
    tlo = [
        nc.dram_tensor(f"tlo{l}", [LO, H], BF16, kind="Internal", addr_space="Shared")
        for l in range(L)
    ]
    thi = [
        nc.dram_tensor(f"thi{l}", [HI, H], BF16, kind="Internal", addr_space="Shared")
        for l in range(L)
    ]
    agin_a = [nc.dram_tensor(f"agina{l}", [128, TSPL, H], BF16, kind="Internal") for l in range(L)]
    agin_b = [nc.dram_tensor(f"aginb{l}", [128, T - TSPL, H], BF16, kind="Internal") for l in range(L)]

    RG = [list(range(M))]

    def ag_a(l):
        nc.gpsimd.collective_compute(
            "AllGather", mybir.AluOpType.bypass, replica_groups=RG,
            ins=[agin_a[l][:]], outs=[tlo[l][:]],
        )

    def ag_b(l):
        nc.gpsimd.collective_compute(
            "AllGather", mybir.AluOpType.bypass, replica_groups=RG,
            ins=[agin_b[l][:]], outs=[thi[l][:]],
        )

    with tile.TileContext(nc) as tc:
        with (
            tc.tile_pool(name="const", bufs=1) as const,
            tc.tile_pool(name="gsb", bufs=5) as gp,
            tc.tile_pool(name="ssb", bufs=2) as sp,
            tc.tile_pool(name="wsb", bufs=2) as sb,
            tc.tile_pool(name="psg", bufs=2, space="PSUM") as psg,
            tc.tile_pool(name="pso", bufs=2, space="PSUM") as pso,
            tc.tile_pool(name="pse", bufs=1, space="PSUM") as pse,
            tc.tile_pool(name="pst", bufs=2, space="PSUM") as pst,
            tc.tile_pool(name="psc", bufs=1, space="PSUM") as psc,
        ):
            ident = const.tile([128, 128], F32)
            make_identity(nc, ident[:])
            ident_bf = const.tile([128, 128], BF16)
            nc.vector.tensor_copy(ident_bf[:], ident[:])
            fc0w_sb = const.tile([D, H], BF16)
            nc.sync.dma_start(fc0w_sb[:], fc0_w[:])
            b0_sb = const.tile([H, 1], F32)
            nc.sync.dma_start(b0_sb[:], b0col[:])
            wtop_sb = [const.tile([H, K * H], BF16, tag=f"wtop{l}", name=f"wtop{l}") for l in range(L)]
            wbot_sb = [const.tile([H, K * H], BF16, tag=f"wbot{l}", name=f"wbot{l}") for l in range(L)]
            envw_sb = [const.tile([H, K], BF16, tag=f"envw{l}", name=f"envw{l}") for l in range(L)]
            envb_sb = [const.tile([128, T, K], F32, tag=f"envb{l}", name=f"envb{l}") for l in range(L)]
            for l in range(L):
                nc.sync.dma_start(wtop_sb[l][:], wtop[l])
                nc.sync.dma_start(wbot_sb[l][:], wbot[l])
                nc.sync.dma_start(envw_sb[l][:], env_w[l])
                nc.sync.dma_start(envb_sb[l][:], envb_io[l])
            fc1w_sb = const.tile([H, C], BF16)
            nc.sync.dma_start(fc1w_sb[:], fc1_w[:])
            b1_sb = const.tile([128, C], F32)
            nc.sync.dma_start(b1_sb[:], b1[:])
            idx_sb = [
                const.tile([128, int(tot[s]) * 8], I16, tag=f"idx{s}", name=f"idxsb{s}") for s in range(2)
            ]
            for s in range(2):
                nc.sync.dma_start(idx_sb[s][:], idx_io[s][:])
            hT_bf = const.tile([H, NPAD], BF16)  # feat-major h (matmul operand)
            h_node = const.tile([128, T, 128], BF16)  # node-major h blocks

            # ---------------- fc0 (4-tile batched) ----------------
            for tb in range(0, T, 4):
                nt = min(4, T - tb)
                xt = sb.tile([D, 512], BF16, tag="xt")
                nc.sync.dma_start(
                    xt[:, : nt * 128], xT[:, tb * 128 : (tb + nt) * 128]
                )
                z4 = pso.tile([128, 512], F32, tag="po")
                for ti in range(nt):
                    nc.tensor.matmul(
                        z4[:, ti * 128 : (ti + 1) * 128],
                        fc0w_sb[:], xt[:, ti * 128 : (ti + 1) * 128],
                        start=True, stop=True,
                    )
                nc.scalar.activation(
                    hT_bf[:, tb * 128 : (tb + nt) * 128], z4[:, : nt * 128],
                    mybir.ActivationFunctionType.Relu, bias=b0_sb[:, 0:1],
                )
                for ti in range(nt):
                    t = tb + ti
                    ztr = pst.tile([128, H], BF16, tag="trb")
                    nc.tensor.transpose(
                        ztr[:], hT_bf[:, t * 128 : (t + 1) * 128], ident_bf[:]
                    )
                    nc.scalar.activation(
                        h_node[:, t, :], ztr[:], mybir.ActivationFunctionType.Copy
                    )
                    if t == TSPL - 1:
                        nc.scalar.dma_start(agin_a[0][:], h_node[:, 0:TSPL, :])
                        ag_a(0)
            nc.scalar.dma_start(agin_b[0][:], h_node[:, TSPL:T, :])
            ag_b(0)

            # ---------------- conv layers ----------------
            qcnt = 0  # global SWDGE queue round-robin counter
            for l in range(L):
                last = l == L - 1

                # Build this layer's gather-call order. Layer 0: both tables
                # land early, interleave lo/hi so the first dest tiles get
                # data fast. Layer l>0: AG-hi(l) still needs the previous
                # layer's last tiles; emit 4 lo calls, then the collective,
                # then interleave the rest so nothing head-of-line blocks.
                def _interleave(a, b):
                    out = []
                    for i in range(max(len(a), len(b))):
                        if i < len(a):
                            out.append(a[i])
                        if i < len(b):
                            out.append(b[i])
                    return out

                lo_calls = [(0, i) for i in range(len(callsz[0]))]
                hi_calls = [(1, i) for i in range(len(callsz[1]))]
                events = {}  # emit-position -> callable
                if l == 0:
                    order = _interleave(lo_calls, hi_calls)
                else:
                    # Pre-gen at most G-pool-depth-1 lo calls during the
                    # previous layer's tail (a deeper head would deadlock:
                    # lo call B+k's WAR wait blocks the in-order issue of
                    # the hi calls its consumers need).
                    head = lo_calls[:4]
                    rest = _interleave(hi_calls, lo_calls[4:])
                    order = head + rest
                    events[len(head)] = lambda l=l: ag_b(l)

                gtiles = [[None] * len(callsz[0]), [None] * len(callsz[1])]
                off = [0, 0]
                # call start offsets per stream
                starts = [np.concatenate([[0], np.cumsum(callsz[s])]) for s in range(2)]
                for pos, (s, g) in enumerate(order):
                    if pos in events:
                        events[pos]()
                    cs = callsz[s][g]
                    o8 = int(starts[s][g]) * 8
                    gt = gp.tile([128, CALL, H], BF16, tag=f"G{s}")
                    src = tlo[l] if s == 0 else thi[l]
                    nc.gpsimd.dma_gather(
                        gt[:, :cs, :],
                        src[:, :],
                        idx_sb[s][:, o8 : o8 + cs * 8],
                        num_idxs=cs * 128,
                        num_idxs_reg=cs * 128,
                        elem_size=H,
                        single_packet=False,
                        queue_num=qcnt % 4,
                    )
                    qcnt += 1
                    gtiles[s][g] = gt
                if len(order) in events:
                    events[len(order)]()

                # streamed S-matrix windows (SWIN chunks each)
                stiles = [[], []]
                for g in range(max(-(-int(tot[0]) // SWIN), -(-int(tot[1]) // SWIN))):
                    for s in range(2):
                        c0 = g * SWIN
                        if c0 < int(tot[s]):
                            cs = min(SWIN, int(tot[s]) - c0)
                            st = sp.tile([128, SWIN, 128], BF16, tag=f"S{s}")
                            nc.sync.dma_start(
                                st[:, :cs, :], s_io[s][:, c0 : c0 + cs, :]
                            )
                            stiles[s].append(st)

                # ---- bulk expert-gate phase (rides the gather shadow) ----
                pe = pse.tile([128, T, K], F32, tag="pe")
                for t in range(T):
                    nc.tensor.matmul(
                        pe[:, t, :],
                        hT_bf[:, t * 128 : (t + 1) * 128], envw_sb[l][:],
                        start=True, stop=True,
                    )
                z_sb = sb.tile([128, T, K], F32, tag="zsb")
                nc.vector.tensor_add(z_sb[:, :, :], pe[:, :, :], envb_sb[l][:, :, :])
                negmx = sb.tile([128, T], F32, tag="negmx")
                nc.vector.reduce_max(
                    negmx[:], z_sb[:, :, :], axis=mybir.AxisListType.X, negate=True
                )
                ea = sb.tile([128, T, K], F32, tag="ea")
                esum = sb.tile([128, T], F32, tag="esum")
                for t in range(T):
                    nc.scalar.activation(
                        ea[:, t, :], z_sb[:, t, :],
                        mybir.ActivationFunctionType.Exp,
                        bias=negmx[:, t : t + 1],
                        accum_out=esum[:, t : t + 1],
                    )
                recip = sb.tile([128, T], F32, tag="recip")
                nc.vector.reciprocal(recip[:], esum[:])

                # ---- per-tile loop, software-pipelined ----
                # Stage A(t): gcn chunk matmuls -> pg psum; gcn_bf copy.
                # Stage B(t): expert matmuls + mix + relu  (emitted at t+1)
                # Stage C(t): transpose + hT copy / fc1    (emitted at t+2)
                # The delays keep each in-order engine queue free of
                # cross-engine round-trip stalls: everything an instruction
                # waits on was emitted >= 1 tile earlier.
                gcn_bfs, pos, ptrs = {}, {}, {}

                def stage_a(t):
                    chunks = []
                    for s in range(2):
                        for j in range(NCH[t, s]):
                            chunks.append((s, int(base[t, s]) + j))
                    pg = psg.tile([H, 128], F32, tag="gcn")
                    for j, (s, ch) in enumerate(chunks):
                        gcall = int(np.searchsorted(starts[s], ch, side="right")) - 1
                        gt = gtiles[s][gcall]
                        st = stiles[s][ch // SWIN]
                        nc.tensor.matmul(
                            pg[:],
                            gt[:, ch - int(starts[s][gcall]), :],
                            st[:, ch % SWIN, :],
                            start=(j == 0),
                            stop=(j == len(chunks) - 1),
                        )
                    gcn_bf = sb.tile([H, 128], BF16, tag="gcnbf")
                    nc.scalar.activation(
                        gcn_bf[:], pg[:], mybir.ActivationFunctionType.Copy
                    )
                    gcn_bfs[t] = gcn_bf

                def stage_b(t):
                    hsl = hT_bf[:, t * 128 : (t + 1) * 128]
                    po = pso.tile([128, K * H], F32, tag="po")
                    nc.tensor.matmul(
                        po[:], gcn_bfs.pop(t)[:], wtop_sb[l][:], start=True, stop=False
                    )
                    nc.tensor.matmul(
                        po[:], hsl, wbot_sb[l][:], start=False, stop=True
                    )
                    # mix: h_new = relu((sum_k ea_k * po_k) * recip)
                    mixs = sb.tile([128, K * H], F32, tag="mixs")
                    nc.scalar.activation(
                        mixs[:, 0:H], po[:, 0:H],
                        mybir.ActivationFunctionType.Copy,
                        scale=ea[:, t, 0:1],
                    )
                    nc.vector.tensor_scalar_mul(
                        mixs[:, H : 2 * H], po[:, H : 2 * H], ea[:, t, 1:2]
                    )
                    nc.scalar.activation(
                        mixs[:, 2 * H : 3 * H], po[:, 2 * H : 3 * H],
                        mybir.ActivationFunctionType.Copy,
                        scale=ea[:, t, 2:3],
                    )
                    nc.vector.tensor_scalar_mul(
                        mixs[:, 3 * H : 4 * H], po[:, 3 * H : 4 * H], ea[:, t, 3:4]
                    )
                    t2 = sb.tile([128, 2 * H], F32, tag="t2")
                    nc.vector.tensor_add(
                        t2[:], mixs[:, 0 : 2 * H], mixs[:, 2 * H : 4 * H]
                    )
                    msum = sb.tile([128, H], F32, tag="msum")
                    nc.vector.tensor_add(msum[:], t2[:, 0:H], t2[:, H : 2 * H])
                    nc.scalar.activation(
                        h_node[:, t, :], msum[:], mybir.ActivationFunctionType.Relu,
                        scale=recip[:, t : t + 1],
                    )
                    if not last:
                        if t == TSPL - 1:
                            nc.scalar.dma_start(agin_a[1][:], h_node[:, 0:TSPL, :])
                            ag_a(1)
                        elif t == T - 1:
                            nc.scalar.dma_start(agin_b[1][:], h_node[:, TSPL:T, :])

                def stage_c(t):
                    ptr = pst.tile([128, H], BF16, tag="trb")
                    nc.tensor.transpose(ptr[:], h_node[:, t, :], ident_bf[:])
                    if not last:
                        nc.scalar.activation(
                            hT_bf[:, t * 128 : (t + 1) * 128], ptr[:],
                            mybir.ActivationFunctionType.Copy,
                        )
                    else:
                        h2T = sb.tile([H, 128], BF16, tag="h2T")
                        nc.scalar.activation(
                            h2T[:], ptr[:], mybir.ActivationFunctionType.Copy
                        )
                        pc = psc.tile([128, C], F32, tag="c")
                        nc.tensor.matmul(
                            pc[:], h2T[:], fc1w_sb[:], start=True, stop=True
                        )
                        ob = sb.tile([128, C], F32, tag="ob")
                        nc.vector.tensor_add(ob[:], pc[:], b1_sb[:])
                        nc.scalar.dma_start(
                            out_io[t * 128 : (t + 1) * 128, :], ob[:]
                        )

                for t in range(T + 2):
                    if 1 <= t <= T:
                        stage_b(t - 1)
                    if t >= 2:
                        stage_c(t - 2)
                    if t < T:
                        stage_a(t)
    nc.compile()
    return nc


def _in_maps(prep):
    maps = []
    for c in range(M):
        m = {
            "xT": prep["xT"][c],
            "fc0_w": prep["fc0_w"],
            "b0col": prep["b0"][:, None].copy(),
            "wtop": prep["wtop"],
            "wbot": prep["wbot"],
            "env_w": prep["env_w_bf"],
            "envb": prep["envb"],
            "fc1_w": prep["fc1_w"],
            "b1": prep["b1_bcast"],
        }
        for s in range(2):
            m[f"idx{s}"] = prep["idx_arr"][c, s]
            m[f"smat{s}"] = prep["s_arr"][c, s]
        maps.append(m)
    return maps


_compiled = {}


def _get_compiled(prep, key):
    if key not in _compiled:
        _compiled[key] = _build_program(prep)
    return _compiled[key]


def kernel(trace=False, **inputs):
    inputs = {k: np.asarray(v) for k, v in inputs.items()}
    prep = _preprocess(**inputs)
    key = hash(inputs["edge_index"].tobytes()) ^ hash(inputs["x"].tobytes()[:4096])
    nc = _get_compiled(prep, key)
    res = bass_utils.run_bass_kernel_spmd(
        nc, _in_maps(prep), core_ids=list(range(M)), trace=trace
    )
    out = np.zeros((N, C), np.float32)
    for c in range(M):
        out[c * NPC : (c + 1) * NPC] = res.results[c]["out"][:NPC]
    kernel.last_exec_time_ns = res.exec_time_ns
    kernel.last_results = res
    return out
